# revision 1
# baseline (speedup 1.0000x reference)
"""DeepSeekMoE layer on 8 TRN2 NeuronCores — expert-parallel with host token dispatch.

Reference computation (per token):
    shared = silu(x @ ws1) @ ws2
    router: softmax(x @ w_router) -> top-2 -> renormalize -> gates
    routed = sum_{e in top2} gate_e * silu(x @ w1[e]) @ w2[e]
    out    = shared + routed

Sharding: expert-parallel. Core e receives the (padded) bucket of all token
rows routed to expert e (capacity C), plus a 1/8 slice of all tokens for the
shared expert. Routing (softmax/top-k) and the dispatch/combine permutations
run on the host; all GEMMs + SiLU + gate scaling run on device.

Per-core device kernel (two passes through a DRAM-resident intermediate):
    pass1: hT = silu(w1e.T @ xT)   [I x R] with I=1408 on partitions
    pass2: y  = hT.T @ w2e         [R x H] scaled per-row by the gate
All matmuls run as float32r (full fp32 storage; TensorE full-rate mode).
"""

import numpy as np
import ml_dtypes

import concourse.mybir as mybir
import concourse.tile as tile
from concourse import bacc
from concourse.bass_utils import run_bass_kernel_spmd

H = 2048          # hidden
I = 1408          # moe intermediate
E = 8             # routed experts == n cores
NCORES = 8
RT = 512          # token tile (columns of xT / rows of y) per step
KH = H // 128     # 16 k-tiles over hidden
KI = 11           # 11 k-tiles over intermediate
F32 = mybir.dt.float32
F32R = mybir.dt.float32r
BF16 = mybir.dt.bfloat16

_BUILD_CACHE: dict = {}

NPRE = 4          # weight k-slices prefetched into wpre for the next phase
IN_BUFS = 4

_STORE_RR = [0]


def _store(nc, dst, src):
    # rotate h/y stores across the gpsimd and sync DMA queues
    eng = (nc.gpsimd, nc.sync)[_STORE_RR[0] % 2]
    _STORE_RR[0] += 1
    eng.dma_start(dst, src)


def _wdma(nc, dst, src, idx):
    # weight streams alternate between the scalar and gpsimd queues
    eng = (nc.scalar, nc.gpsimd)[idx % 2]
    eng.dma_start(dst, src)


def _silu_evict(nc, out_pool, ps, tag_id, use_silu, sz=RT):
    ev = out_pool.tile([128, RT], F32R, tag="ev", bufs=4, name=f"ev_{tag_id}")
    if use_silu:
        nc.scalar.activation(ev[:, :sz], ps[:], mybir.ActivationFunctionType.Silu)
    else:
        sg = out_pool.tile([128, RT], F32, tag="sg", bufs=2, name=f"sg_{tag_id}")
        nc.scalar.activation(sg[:, :sz], ps[:], mybir.ActivationFunctionType.Sigmoid)
        nc.vector.tensor_mul(ev[:, :sz], ps[:], sg[:, :sz])
    return ev


def _tiles(ncols):
    """R-tile (offset, size) list: 512-tiles plus an optional 256 tail."""
    out, off = [], 0
    while ncols - off >= RT:
        out.append((off, RT))
        off += RT
    if ncols - off:
        assert (ncols - off) % 256 == 0
        out.append((off, ncols - off))
        off = ncols
    return out


def _emit_pass1(nc, pools, wpool, w_dram, w_pre, xt, ht, ncols, use_silu, ph):
    """ht[:, i, :] = silu(w.T @ xT) — I on partitions, tokens on free.

    First R-tile: per-k sliced stream+weight DMAs in consumption order and a
    k-outer chunked matmul loop, so the PE consumes slices as they arrive.
    Later R-tiles: i-outer / k-inner with whole-half stream DMAs.
    """
    in_pool, out_pool, psum_pool = pools
    npre = len(w_pre) if w_pre else 0

    # --- first R-tile: sliced, streaming ---
    cs = slice(0, RT)
    xh0 = in_pool.tile([128, 8, RT], F32R, tag="sin", name=f"xh0_{ph}_0")
    xh1 = in_pool.tile([128, 8, RT], F32R, tag="sin", name=f"xh1_{ph}_0")
    w = wpool.tile([128, KH, I], F32R, tag="w", name=f"w_{ph}")

    def xslice(k):
        return (xh0 if k < 8 else xh1)[:, k % 8, :]

    if npre:
        # stream slices first (prefetchable), then the WAR-blocked weight rest
        for k in range(KH):
            nc.sync.dma_start(xslice(k), xt[:, k, cs])
        for k in range(npre, KH):
            _wdma(nc, w[:, k, :], w_dram[:, k, :], k)
    else:
        # cold start: interleave in consumption order
        for k in range(KH):
            nc.sync.dma_start(xslice(k), xt[:, k, cs])
            _wdma(nc, w[:, k, :], w_dram[:, k, :], k)

    for lo, hi in ((0, 6), (6, KI)):
        pss = [psum_pool.tile([128, RT], F32, tag="ps", name=f"ps_{ph}_0_{i}")
               for i in range(lo, hi)]
        for k in range(KH):
            w_k = w_pre[k] if k < npre else w[:, k, :]
            for i in range(lo, hi):
                nc.tensor.matmul(
                    pss[i - lo][:], w_k[:, i * 128:(i + 1) * 128], xslice(k),
                    start=(k == 0), stop=(k == KH - 1))
        for glo in range(lo, hi, 3):
            ghi = min(glo + 3, hi)
            evg = out_pool.tile([128, 3, RT], F32R, tag="ev", bufs=2,
                                name=f"evg_{ph}_0_{glo}")
            for i in range(glo, ghi):
                nc.scalar.activation(evg[:, i - glo, :], pss[i - lo][:],
                                     mybir.ActivationFunctionType.Silu)
            _store(nc, ht[:, glo:ghi, cs], evg[:, :ghi - glo, :])
    if npre:
        # also load the wpre-covered slices into the main tile for later R-tiles
        for k in range(npre):
            _wdma(nc, w[:, k, :], w_dram[:, k, :], k)

    next_pre = None

    # --- remaining R-tiles (maybe a 256-wide tail) ---
    for r, (off, sz) in enumerate(_tiles(ncols)):
        if r == 0:
            continue
        cs = slice(off, off + sz)
        xh0 = in_pool.tile([128, 8, sz], F32R, tag="sin", name=f"xh0_{ph}_{r}")
        xh1 = in_pool.tile([128, 8, sz], F32R, tag="sin", name=f"xh1_{ph}_{r}")
        nc.sync.dma_start(xh0[:], xt[:, 0:8, cs])
        nc.sync.dma_start(xh1[:], xt[:, 8:16, cs])
        for glo in range(0, KI, 3):
            ghi = min(glo + 3, KI)
            evg = out_pool.tile([128, 3, RT], F32R, tag="ev", bufs=2,
                                name=f"evg_{ph}_{r}_{glo}")
            for i in range(glo, ghi):
                ps = psum_pool.tile([128, sz], F32, tag="ps",
                                    name=f"ps_{ph}_{r}_{i}")
                for k in range(KH):
                    nc.tensor.matmul(
                        ps[:], w[:, k, i * 128:(i + 1) * 128],
                        (xh0 if k < 8 else xh1)[:, k % 8, :],
                        start=(k == 0), stop=(k == KH - 1))
                nc.scalar.activation(evg[:, i - glo, :sz], ps[:],
                                     mybir.ActivationFunctionType.Silu)
            _store(nc, ht[:, glo:ghi, cs], evg[:, :ghi - glo, :sz])
        if r == 1:
            next_pre = yield  # build() emits the next phase's wpre DMAs here
    if len(_tiles(ncols)) < 2:
        next_pre = yield
    yield next_pre


def _emit_pass2(nc, pools, wpool, w_dram, w_pre, ht, y, ncols, scale_sb, ph):
    """y[m, :] = (hT.T @ w2) * gate[m] — tokens on partitions, bf16 out.

    First R-tile: per-i sliced DMAs + i-outer over two chunks of 8 psum banks.
    Later R-tiles: i-inner. Evictions group 4 H-blocks into one [128, H] bf16
    buffer per 128-token block, stored with a single DMA.
    """
    in_pool, out_pool, psum_pool = pools
    npre = len(w_pre) if w_pre else 0

    def evict(ps, yb, m, hblk):
        yt = yb[:, hblk * 512:(hblk + 1) * 512]
        if scale_sb is not None:
            nc.vector.tensor_scalar_mul(yt, ps[:], scale_sb[:, m:m + 1])
        else:
            nc.vector.tensor_copy(yt, ps[:])

    def ybuf(m):
        return out_pool.tile([128, H], BF16, tag="yb", bufs=2, name=f"yb_{ph}_{m}")

    # --- first R-tile: sliced, streaming ---
    cs = slice(0, RT)
    hh0 = in_pool.tile([128, 6, RT], F32R, tag="sin", name=f"hh0_{ph}_0")
    hh1 = in_pool.tile([128, 5, RT], F32R, tag="sin", name=f"hh1_{ph}_0")
    w = wpool.tile([128, KI, H], F32R, tag="w", name=f"w_{ph}")

    def hslice(i):
        return (hh0 if i < 6 else hh1)[:, i if i < 6 else i - 6, :]

    def lhs(i, c):
        return hslice(i)[:, c * 128:(c + 1) * 128]

    for i in range(KI):
        nc.sync.dma_start(hslice(i), ht[:, i, cs])
    for i in range(npre, KI):
        _wdma(nc, w[:, i, :], w_dram[:, i, :], i)

    pairs = [(c, hb) for c in range(RT // 128) for hb in range(H // 512)]
    for chunk in (pairs[:8], pairs[8:]):
        pss = {p: psum_pool.tile([128, 512], F32, tag="ps",
                                 name=f"ps_{ph}_0_{p[0]}_{p[1]}")
               for p in chunk}
        for i in range(KI):
            w_i = w_pre[i] if i < npre else w[:, i, :]
            for (c, hb) in chunk:
                nc.tensor.matmul(
                    pss[(c, hb)][:], lhs(i, c), w_i[:, hb * 512:(hb + 1) * 512],
                    start=(i == 0), stop=(i == KI - 1))
        ybs = {c: ybuf(c) for c in {c for c, _ in chunk}}
        for (c, hb) in chunk:
            evict(pss[(c, hb)], ybs[c], c, hb)
        for c, yb in ybs.items():
            _store(nc, y[:, c, :], yb[:])
    for i in range(npre):
        _wdma(nc, w[:, i, :], w_dram[:, i, :], i)

    next_pre = None

    # --- remaining R-tiles (maybe a 256-wide tail) ---
    for r, (off, sz) in enumerate(_tiles(ncols)):
        if r == 0:
            continue
        cs = slice(off, off + sz)
        hh0 = in_pool.tile([128, 6, sz], F32R, tag="sin", name=f"hh0_{ph}_{r}")
        hh1 = in_pool.tile([128, 5, sz], F32R, tag="sin", name=f"hh1_{ph}_{r}")
        nc.sync.dma_start(hh0[:], ht[:, 0:6, cs])
        nc.sync.dma_start(hh1[:], ht[:, 6:KI, cs])
        for c in range(sz // 128):
            m = off // 128 + c
            yb = ybuf(m)
            for hb in range(H // 512):
                ps = psum_pool.tile([128, 512], F32, tag="ps", name=f"ps_{ph}_{m}_{hb}")
                for i in range(KI):
                    src = hh0 if i < 6 else hh1
                    j = i if i < 6 else i - 6
                    nc.tensor.matmul(
                        ps[:], src[:, j, c * 128:(c + 1) * 128],
                        w[:, i, hb * 512:(hb + 1) * 512],
                        start=(i == 0), stop=(i == KI - 1))
                evict(ps, yb, m, hb)
            _store(nc, y[:, m, :], yb[:])
        if r == 1:
            next_pre = yield
    if len(_tiles(ncols)) < 2:
        next_pre = yield
    yield next_pre


def build(C, S, debug=False, use_silu=True, reps=1):
    """Build the per-core Bass module. C: expert capacity, S: shared rows.

    reps>1 repeats the whole computation in one NEFF (timing use only)."""
    assert C % 256 == 0 and C >= RT and S % RT == 0
    _STORE_RR[0] = 0
    nc = bacc.Bacc(None, target_bir_lowering=False, debug=debug)
    with tile.TileContext(nc) as tc:
        with tc.tile_pool(name="dram", bufs=1, space="DRAM") as dram:
            xtd = dram.tile((128, KH, C), F32R, kind="ExternalInput", name="xtd", uniquify=False)
            xts = dram.tile((128, KH, S), F32R, kind="ExternalInput", name="xts", uniquify=False)
            w1e = dram.tile((128, KH, I), F32R, kind="ExternalInput", name="w1e", uniquify=False)
            w2e = dram.tile((128, KI, H), F32R, kind="ExternalInput", name="w2e", uniquify=False)
            ws1 = dram.tile((128, KH, I), F32R, kind="ExternalInput", name="ws1", uniquify=False)
            ws2 = dram.tile((128, KI, H), F32R, kind="ExternalInput", name="ws2", uniquify=False)
            gate = dram.tile((128, C // 128), F32, kind="ExternalInput", name="gate", uniquify=False)
            yd = dram.tile((128, C // 128, H), BF16, kind="ExternalOutput", name="yd", uniquify=False)
            ys = dram.tile((128, S // 128, H), BF16, kind="ExternalOutput", name="ys", uniquify=False)
            htd = dram.tile((128, KI, C), F32R, name="htd", uniquify=False)
            hts = dram.tile((128, KI, S), F32R, name="hts", uniquify=False)

            with (
                tc.tile_pool(name="wpool", bufs=1) as wpool,
                tc.tile_pool(name="wpre", bufs=NPRE) as wpre_pool,
                tc.tile_pool(name="inpool", bufs=IN_BUFS) as in_pool,
                tc.tile_pool(name="outpool", bufs=6) as out_pool,
                tc.tile_pool(name="psum", bufs=8, space="PSUM") as psum_pool,
                tc.tile_pool(name="const", bufs=1) as const_pool,
            ):
                pools = (in_pool, out_pool, psum_pool)
                scale_sb = const_pool.tile([128, C // 128], F32, name="scale_sb")
                nc.sync.dma_start(scale_sb[:], gate[:])

                def load_pre(dram_w, tag_id):
                    pre = []
                    for k in range(NPRE):
                        t = wpre_pool.tile([128, H], F32R, tag="wpre",
                                           name=f"wpre_{tag_id}_{k}")
                        tv = t[:, :dram_w.shape[2]]
                        _wdma(nc, tv, dram_w[:, k, :], k)
                        pre.append(tv)
                    return pre

                pre_a = None
                for rep in range(reps):
                    gen = _emit_pass1(nc, pools, wpool, w1e, pre_a, xtd, htd, C,
                                      use_silu, f"a{rep}")
                    next(gen)
                    pre_c = gen.send(load_pre(w2e, f"c{rep}"))

                    gen = _emit_pass2(nc, pools, wpool, w2e, pre_c, htd, yd, C,
                                      scale_sb, f"c{rep}")
                    next(gen)
                    pre_b = gen.send(load_pre(ws1, f"b{rep}"))

                    gen = _emit_pass1(nc, pools, wpool, ws1, pre_b, xts, hts, S,
                                      use_silu, f"b{rep}")
                    next(gen)
                    pre_d = gen.send(load_pre(ws2, f"d{rep}"))

                    gen = _emit_pass2(nc, pools, wpool, ws2, pre_d, hts, ys, S,
                                      None, f"d{rep}")
                    next(gen)
                    pre_a = gen.send(load_pre(w1e, f"a{rep + 1}")
                                     if rep + 1 < reps else None)

    nc.compile()
    return nc


def _get_built(C, S):
    key = (C, S)
    if key not in _BUILD_CACHE:
        _BUILD_CACHE[key] = build(C, S)
    return _BUILD_CACHE[key]


def _to_kxm_layout(a):
    """[K, M] -> [128, K/128, M] with logical row k at (k%128, k//128)."""
    k, m_ = a.shape
    return np.ascontiguousarray(a.reshape(k // 128, 128, m_).transpose(1, 0, 2))


def _round_fp32r(a):
    """Round fp32 to the fp32r grid (RNE to 1s+8e+11m; low 12 bits zero)."""
    u = np.ascontiguousarray(a).view(np.uint32)
    lsb = (u >> 12) & 1
    return ((u + 0x7FF + lsb) & 0xFFFFF000).view(np.float32)


def route_and_dispatch(xf, w_router):
    """Host router: returns (sorted token ids, gates, per-expert offsets, capacity)."""
    T = xf.shape[0]
    logits = xf @ w_router                       # [T, E]
    order = np.argsort(-logits, axis=1, kind="stable")[:, :2]
    mx = logits.max(axis=1, keepdims=True)
    p = np.exp(logits - mx)
    p /= p.sum(axis=1, keepdims=True)
    tk = np.take_along_axis(p, order, axis=1)    # [T, 2]
    g = tk / tk.sum(axis=1, keepdims=True)

    pe = order.ravel()                           # expert id per (token, slot) pair
    ptok = np.repeat(np.arange(T, dtype=np.int64), 2)
    pg = g.astype(np.float32).ravel()
    perm = np.argsort(pe, kind="stable")
    stok, sg = ptok[perm], pg[perm]
    counts = np.bincount(pe, minlength=E)
    offs = np.zeros(E + 1, dtype=np.int64)
    np.cumsum(counts, out=offs[1:])
    C = max(512, int(-(-counts.max() // 256) * 256))
    return stok, sg, offs, C


def prepare(x, w_shared1, w_shared2, w1, w2, w_router):
    """Host-side routing + dispatch. Returns (in_maps, meta)."""
    x = np.asarray(x, dtype=np.float32)
    w_shared1 = np.asarray(w_shared1, dtype=np.float32)
    w_shared2 = np.asarray(w_shared2, dtype=np.float32)
    w1 = np.asarray(w1, dtype=np.float32)
    w2 = np.asarray(w2, dtype=np.float32)
    w_router = np.asarray(w_router, dtype=np.float32)

    B, Sq, _ = x.shape
    T = B * Sq
    S = T // NCORES                              # shared-expert rows per core
    xf = x.reshape(T, H)

    stok, sg, offs, C = route_and_dispatch(xf, w_router)

    # pre-round matmul operands to the fp32r grid (router used unrounded xf)
    xf = _round_fp32r(xf)
    ws1_l = _to_kxm_layout(_round_fp32r(w_shared1))
    ws2_l = _to_kxm_layout(_round_fp32r(w_shared2))
    w1 = _round_fp32r(w1)
    w2 = _round_fp32r(w2)

    in_maps = []
    for e in range(NCORES):
        toks = stok[offs[e]:offs[e + 1]]
        n = len(toks)
        xd = np.zeros((C, H), np.float32)
        xd[:n] = xf[toks]
        gate_v = np.zeros(C, np.float32)
        gate_v[:n] = sg[offs[e]:offs[e + 1]]
        xs = xf[e * S:(e + 1) * S]
        in_maps.append({
            "xtd": np.ascontiguousarray(xd.reshape(C, KH, 128).transpose(2, 1, 0)),
            "xts": np.ascontiguousarray(xs.reshape(S, KH, 128).transpose(2, 1, 0)),
            "w1e": _to_kxm_layout(w1[e]),
            "w2e": _to_kxm_layout(w2[e]),
            "ws1": ws1_l,
            "ws2": ws2_l,
            "gate": np.ascontiguousarray(gate_v.reshape(C // 128, 128).T),
        })

    meta = (B, Sq, T, S, C, stok, offs)
    return in_maps, meta


def combine(results, meta):
    """Host-side gather/unshard of per-core outputs to the full output."""
    B, Sq, T, S, C, stok, offs = meta
    out = np.zeros((T, H), np.float32)
    for e in range(NCORES):
        toks = stok[offs[e]:offs[e + 1]]
        ydp = np.asarray(results[e]["yd"], dtype=np.float32
                         ).transpose(1, 0, 2).reshape(C, H)
        out[toks] += ydp[:len(toks)]
        ysp = np.asarray(results[e]["ys"], dtype=np.float32
                         ).transpose(1, 0, 2).reshape(S, H)
        out[e * S:(e + 1) * S] += ysp
    return out.reshape(B, Sq, H)


def kernel(x, w_shared1, w_shared2, w1, w2, w_router):
    in_maps, meta = prepare(x, w_shared1, w_shared2, w1, w2, w_router)
    C, S = meta[4], meta[3]
    nc = _get_built(C, S)
    res = run_bass_kernel_spmd(nc, in_maps, core_ids=list(range(NCORES)))
    return combine(res.results, meta)



# revision 2
# speedup vs baseline: 1.1012x; 1.1012x over previous
"""DeepSeekMoE layer on 8 TRN2 NeuronCores — expert-parallel, fused MLP.

Reference computation (per token):
    shared = silu(x @ ws1) @ ws2
    router: softmax(x @ w_router) -> top-2 -> renormalize -> gates
    routed = sum_{e in top2} gate_e * silu(x @ w1[e]) @ w2[e]
    out    = shared + routed

Sharding: expert-parallel. Core e receives the (padded to 128) bucket of all
token rows routed to expert e (capacity C), plus a 1/8 slice of all tokens
for the shared expert (S rows), packed into one [128, 16, C+S] input. Routing
(softmax/top-k) and the dispatch/combine permutations run on the host; all
GEMMs + SiLU + gate scaling run on device.

Device kernel: for each 512-token tile, both GEMMs run back to back with the
intermediate h = silu(x @ w1) kept in SBUF (no DRAM round trip). All matmul
operands are bf16 (same 1 cycle/row PE rate as fp32r, half the DMA traffic);
accumulation is fp32 in PSUM. Both weight matrices of the active expert stay
SBUF-resident; the four per-rep weight sets rotate through 3 SBUF slots, with
each load emitted at the program point where its WAR hazard clears so the
issuing queue never head-blocks.
"""

import numpy as np
import ml_dtypes

import concourse.mybir as mybir
import concourse.tile as tile
from concourse import bacc
from concourse.bass_utils import run_bass_kernel_spmd

H = 2048          # hidden
I = 1408          # moe intermediate
E = 8             # routed experts == n cores
NCORES = 8
RT = 512          # token tile (free dim of pass1 / partition chunks of pass2)
KH = H // 128     # 16 k-tiles over hidden
KI = 11           # 11 k-tiles over intermediate
F32 = mybir.dt.float32
BF16 = mybir.dt.bfloat16

_BUILD_CACHE: dict = {}


def _tiles(ncols):
    """R-tile (offset, size) list: 512-tiles plus an optional 128/256/384 tail."""
    out, off = [], 0
    while ncols - off >= RT:
        out.append((off, RT))
        off += RT
    if ncols - off:
        assert (ncols - off) % 128 == 0
        out.append((off, ncols - off))
    return out


def _emit_phase(nc, pools, w1_sb, w2_sb, xt, y, scale_sb, col_off, ncols, ph,
                post_p1_hook=None):
    """One expert phase: y[rows] = silu(x[rows] @ w1) @ w2 * gate[rows].

    Per R-tile: pass1 accumulates hT [I x R] (I on partitions) over 16
    hidden k-slices in two PSUM groups (6+5 banks), evicting through SiLU
    into an SBUF-resident bf16 h tile; pass2 immediately consumes h as lhsT
    to produce y [R x H] (tokens on partitions) in chunks of 8 PSUM banks,
    evicted with the per-token gate scale into bf16 and stored.

    post_p1_hook() is invoked right after the LAST tile's pass1 evictions —
    the point where w1_sb's final read is in flight, so a weight DMA emitted
    there into w1_sb's rotation slot never blocks the queue on its WAR.
    """
    in_pool, psum_pool = pools
    tiles = _tiles(ncols)
    hook_ret = None
    for ti, (off, sz) in enumerate(tiles):
        g0 = col_off + off
        cs = slice(g0, g0 + sz)
        x_sb = in_pool.tile([128, KH, RT], BF16, tag="x", bufs=2,
                            name=f"x_{ph}_{ti}")
        nc.sync.dma_start(x_sb[:, 0:8, :sz], xt[:, 0:8, cs])
        nc.sync.dma_start(x_sb[:, 8:16, :sz], xt[:, 8:16, cs])
        h_sb = in_pool.tile([128, KI, RT], BF16, tag="h", bufs=1,
                            name=f"h_{ph}_{ti}")

        # ---- pass1: hT = silu(w1.T @ xT), I on partitions ----
        for lo, hi in ((0, 6), (6, KI)):
            pss = [psum_pool.tile([128, sz], F32, tag="ps",
                                  name=f"ps1_{ph}_{ti}_{i}")
                   for i in range(lo, hi)]
            for k in range(KH):
                for i in range(lo, hi):
                    nc.tensor.matmul(
                        pss[i - lo][:], w1_sb[:, k, i * 128:(i + 1) * 128],
                        x_sb[:, k, :sz], start=(k == 0), stop=(k == KH - 1))
            for i in range(lo, hi):
                nc.scalar.activation(h_sb[:, i, :sz], pss[i - lo][:],
                                     mybir.ActivationFunctionType.Silu)
        if ti == len(tiles) - 1 and post_p1_hook is not None:
            hook_ret = post_p1_hook()

        # ---- pass2: y = (hT.T @ w2) * gate, tokens on partitions ----
        pairs = [(c, hb) for c in range(sz // 128) for hb in range(H // 512)]
        for chunk in (pairs[j:j + 8] for j in range(0, len(pairs), 8)):
            pss = {p: psum_pool.tile([128, 512], F32, tag="ps",
                                     name=f"ps2_{ph}_{ti}_{p[0]}_{p[1]}")
                   for p in chunk}
            for i in range(KI):
                for c, hb in chunk:
                    nc.tensor.matmul(
                        pss[(c, hb)][:], h_sb[:, i, c * 128:(c + 1) * 128],
                        w2_sb[:, i, hb * 512:(hb + 1) * 512],
                        start=(i == 0), stop=(i == KI - 1))
            cset = sorted({c for c, _ in chunk})
            ybs = {c: in_pool.tile([128, H], BF16, tag="y", bufs=4,
                                   name=f"yb_{ph}_{ti}_{c}")
                   for c in cset}
            for c, hb in chunk:
                m = g0 // 128 + c
                nc.vector.tensor_scalar_mul(
                    ybs[c][:, hb * 512:(hb + 1) * 512], pss[(c, hb)][:],
                    scale_sb[:, m:m + 1])
            for c in cset:
                nc.gpsimd.dma_start(y[:, g0 // 128 + c, :], ybs[c][:])
    return hook_ret


def build(C, S, debug=False, use_silu=True, reps=1):
    """Build the per-core Bass module. C: expert capacity, S: shared rows.

    reps>1 repeats the whole computation in one NEFF (timing use only)."""
    assert C % 128 == 0 and S % RT == 0
    R = C + S
    nc = bacc.Bacc(None, target_bir_lowering=False, debug=debug)
    with tile.TileContext(nc) as tc:
        with tc.tile_pool(name="dram", bufs=1, space="DRAM") as dram:
            xt = dram.tile((128, KH, R), BF16, kind="ExternalInput", name="xt", uniquify=False)
            w1e = dram.tile((128, KH, I), BF16, kind="ExternalInput", name="w1e", uniquify=False)
            w2e = dram.tile((128, KI, H), BF16, kind="ExternalInput", name="w2e", uniquify=False)
            ws1 = dram.tile((128, KH, I), BF16, kind="ExternalInput", name="ws1", uniquify=False)
            ws2 = dram.tile((128, KI, H), BF16, kind="ExternalInput", name="ws2", uniquify=False)
            gate = dram.tile((128, R // 128), F32, kind="ExternalInput", name="gate", uniquify=False)
            y = dram.tile((128, R // 128, H), BF16, kind="ExternalOutput", name="y", uniquify=False)

            with (
                tc.tile_pool(name="wpool", bufs=3) as wpool,
                tc.tile_pool(name="inpool", bufs=2) as in_pool,
                tc.tile_pool(name="psum", bufs=8, space="PSUM") as psum_pool,
                tc.tile_pool(name="const", bufs=1) as const_pool,
            ):
                pools = (in_pool, psum_pool)
                scale_sb = const_pool.tile([128, R // 128], F32, name="scale_sb")
                nc.sync.dma_start(scale_sb[:], gate[:])

                def load_w(dram_w, nk, nm):
                    # weight loads ride the Activation queue, k-sliced so the
                    # PE can consume slices as they stream in
                    t = wpool.tile([128, nk, dram_w.shape[2]], BF16, tag="w",
                                   name=nm)
                    for k in range(nk):
                        nc.scalar.dma_start(t[:, k, :], dram_w[:, k, :])
                    return t

                # 3-slot rotation: per rep the four sets allocate in order
                # w1e, w2e, ws1, ws2 -> slots cycle 0,1,2,0,1,2..., and every
                # load is emitted exactly where its WAR on the evicted set
                # clears (see hooks below).
                w1s = load_w(w1e, KH, "w1s_0")
                w2s = load_w(w2e, KI, "w2s_0")
                for rep in range(reps):
                    ws1s = load_w(ws1, KH, f"ws1s_{rep}")
                    # routed phase; after its last pass1 (w1s dead) load ws2
                    ws2s = _emit_phase(
                        nc, pools, w1s, w2s, xt, y, scale_sb, 0, C,
                        f"r{rep}",
                        post_p1_hook=lambda: load_w(ws2, KI, f"ws2s_{rep}"))
                    # shared phase; at start w2s is dead -> next rep's w1;
                    # after its last pass1 ws1s is dead -> next rep's w2
                    w1s = (load_w(w1e, KH, f"w1s_{rep + 1}")
                           if rep + 1 < reps else None)
                    w2s = _emit_phase(
                        nc, pools, ws1s, ws2s, xt, y, scale_sb, C, S,
                        f"s{rep}",
                        post_p1_hook=(
                            (lambda: load_w(w2e, KI, f"w2s_{rep + 1}"))
                            if rep + 1 < reps else None))

    nc.compile()
    return nc


def _get_built(C, S):
    key = (C, S)
    if key not in _BUILD_CACHE:
        _BUILD_CACHE[key] = build(C, S)
    return _BUILD_CACHE[key]


def _to_kxm_layout(a):
    """[K, M] -> [128, K/128, M] with logical row k at (k%128, k//128)."""
    k, m_ = a.shape
    return np.ascontiguousarray(a.reshape(k // 128, 128, m_).transpose(1, 0, 2))


def route_and_dispatch(xf, w_router):
    """Host router: returns (sorted token ids, gates, per-expert offsets, capacity)."""
    T = xf.shape[0]
    logits = xf @ w_router                       # [T, E]
    order = np.argsort(-logits, axis=1, kind="stable")[:, :2]
    mx = logits.max(axis=1, keepdims=True)
    p = np.exp(logits - mx)
    p /= p.sum(axis=1, keepdims=True)
    tk = np.take_along_axis(p, order, axis=1)    # [T, 2]
    g = tk / tk.sum(axis=1, keepdims=True)

    pe = order.ravel()                           # expert id per (token, slot) pair
    ptok = np.repeat(np.arange(T, dtype=np.int64), 2)
    pg = g.astype(np.float32).ravel()
    perm = np.argsort(pe, kind="stable")
    stok, sg = ptok[perm], pg[perm]
    counts = np.bincount(pe, minlength=E)
    offs = np.zeros(E + 1, dtype=np.int64)
    np.cumsum(counts, out=offs[1:])
    C = max(512, int(-(-counts.max() // 128) * 128))
    return stok, sg, offs, C


def prepare(x, w_shared1, w_shared2, w1, w2, w_router):
    """Host-side routing + dispatch. Returns (in_maps, meta)."""
    x = np.asarray(x, dtype=np.float32)
    w_router = np.asarray(w_router, dtype=np.float32)

    B, Sq, _ = x.shape
    T = B * Sq
    S = T // NCORES                              # shared-expert rows per core
    xf = x.reshape(T, H)

    stok, sg, offs, C = route_and_dispatch(xf, w_router)
    R = C + S

    bf = ml_dtypes.bfloat16
    xb = xf.astype(bf)
    w1b = np.asarray(w1, dtype=np.float32).astype(bf)
    w2b = np.asarray(w2, dtype=np.float32).astype(bf)
    ws1_l = _to_kxm_layout(np.asarray(w_shared1, np.float32).astype(bf))
    ws2_l = _to_kxm_layout(np.asarray(w_shared2, np.float32).astype(bf))

    in_maps = []
    for e in range(NCORES):
        toks = stok[offs[e]:offs[e + 1]]
        n = len(toks)
        xd = np.zeros((R, H), bf)
        xd[:n] = xb[toks]
        xd[C:] = xb[e * S:(e + 1) * S]
        gate_v = np.zeros(R, np.float32)
        gate_v[:n] = sg[offs[e]:offs[e + 1]]
        gate_v[C:] = 1.0
        in_maps.append({
            "xt": np.ascontiguousarray(xd.reshape(R, KH, 128).transpose(2, 1, 0)),
            "w1e": _to_kxm_layout(w1b[e]),
            "w2e": _to_kxm_layout(w2b[e]),
            "ws1": ws1_l,
            "ws2": ws2_l,
            "gate": np.ascontiguousarray(gate_v.reshape(R // 128, 128).T),
        })

    meta = (B, Sq, T, S, C, stok, offs)
    return in_maps, meta


def combine(results, meta):
    """Host-side gather/unshard of per-core outputs to the full output."""
    B, Sq, T, S, C, stok, offs = meta
    out = np.zeros((T, H), np.float32)
    for e in range(NCORES):
        toks = stok[offs[e]:offs[e + 1]]
        yp = np.asarray(results[e]["y"], dtype=np.float32
                        ).transpose(1, 0, 2).reshape(C + S, H)
        out[toks] += yp[:len(toks)]
        out[e * S:(e + 1) * S] += yp[C:]
    return out.reshape(B, Sq, H)


def kernel(x, w_shared1, w_shared2, w1, w2, w_router):
    in_maps, meta = prepare(x, w_shared1, w_shared2, w1, w2, w_router)
    C, S = meta[4], meta[3]
    nc = _get_built(C, S)
    res = run_bass_kernel_spmd(nc, in_maps, core_ids=list(range(NCORES)))
    return combine(res.results, meta)
